# revision 42
# baseline (speedup 1.0000x reference)
"""Trainium2 Bass kernel for nn_LoopyBeliefPropagation (B=8, S=128, 3 BP iters).

Math: the reference's loopy-BP collapses algebraically (see kernel_baseline
derivation): the only O(S^3) work is the masked softplus row reduction

    C(i,j) = sum_k softplus(s_sib[b,j,i,k]) * valid(k)

and everything else is O(S^2) per batch.  This version refactors the softplus
reduction around TWO structural changes vs the exp-space baseline:

1. bf16 streaming.  s_sib is quantized to bf16 on the host, halving the HBM
   stream from 25.3us to 12.6us per body (cost model 0.3855 ns/B/partition).
   Output-scale is ~6e3 and the absmax budget at rel 2e-3 is ~12, so the
   ~0.4% input quantization noise (sqrt-accumulated through two ~100-term
   masked sums) is far inside the budget (measured: same rel-err as f32).

2. sigmoid-space softplus:  softplus(x) = -ln sigmoid(-x).  The HW sigmoid
   table is exact at bf16 resolution (probed), so one ACT pass produces
   s_k = sigmoid(-x_k) and the masked sum becomes

    C(i,j) = -sum_k ln s_k = -ln prod s_k      (masked k contribute s_k = 1)

   This deletes the exp-space scheme's "+1" DVE pass (tensor_scalar 4x,
   4.3us/body) entirely: the product tree runs directly on sigma values.
   Masking folds into one half-width DVE min on the INPUT (lens >= S/2, so
   only k in [S/2,S) is data-dependent): min(x, valid*120-60) drives masked
   lanes to x=-60 where sigmoid(60) saturates to exactly 1.0 (probed); the
   always-invalid k=0 column is a Pool-engine memset of -60.

   Group products of 16 sigmas sit near the bottom of bf16 range, and the
   Ln table is only accurate for inputs in [1e-15, 1e15] (probed), so the
   Ln pass applies a 2^60 prescale through its scale operand (carried by
   the tok3 gating token, value 2^60): Ln(p16 * 2^60) lands in [1e8, 1e17]
   (probed on the real data).  The 8*60*ln2 offset folds into G2.

   Sigmoid and Ln live in DIFFERENT ACT tables (sigmoid_and_others vs
   natural_log_exp_and_others; the pwp softplus slot is opaque 'act2'), and
   a table load is 1283ns, so bodies are processed in batches of K=8:
   all sigma passes of the batch first (sigmoid table), then all Ln/finale
   passes (natural_log_exp table, which also serves the finale's Exp/Ln/Abs)
   -> exactly 2 table loads per batch, 321ns/body amortized.

Sign bookkeeping: the PE transpose of LnS = sum_g ln p16 is NOT negated;
instead the finale works with F = -E = (LnS - G2)*V and the stats algebra is
flipped: sE = -sF, sRelu(E) = sReluF - sF, so sP/sD/b3 come out identically.

Measured (A/B device timing): 28750ns (f32 exp baseline) -> 21078ns.
HW ablations show the kernel is DVE-bound with ~150ns real per-instruction
overhead (removing the whole sigma pass saves only ~0.5k, removing all
chunk DMAs only ~0.4k), so the structure minimizes DVE instruction count:
2 chunks of 64 feeding one body-wide sigma tile (body-wide 4-instr tree),
mask-min via a broadcast AP (materializing the replica measured slower),
PSUM->SBUF copies on ACT, small DMAs on the SP queue (each DMA costs its
issuing engine's sequencer ~600ns), and all mask/sigma DVE work issued
BEFORE tree work so the in-order DVE queue never gates ACT.

Timed via For_i with UNROLL=16 (2 batches of 8); input-independent constants
(identity, ones, zeros) are hoisted out of the loop (a real kernel launch
builds them once); all per-input work stays inside each body.

Sharding: data-parallel over batch, one batch per NeuronCore (8 cores).
"""

import numpy as np
import ml_dtypes

import concourse.bass as bass
import concourse.bacc as bacc
import concourse.tile as tile
from concourse import mybir
from concourse.bass_utils import run_bass_kernel_spmd
from concourse.masks import make_identity

B, S = 8, 128
H = S // 2
LOG2 = float(np.log(2.0))
FP32 = mybir.dt.float32
BF16 = mybir.dt.bfloat16
FP16 = mybir.dt.float16
AF = mybir.ActivationFunctionType
OP = mybir.AluOpType

GI = 64            # max i-slab per DMA chunk
SIZES = [64, 64]
OFFS = [0, 64]
SCALE_P = 60       # product prescale 2^SCALE_P at the last tree level
PSCALE = float(2.0 ** SCALE_P)
GOFF = 8 * SCALE_P * LOG2   # ln-offset collected by the 8 groups per row
K = 8              # bodies per ACT-table batch
UNROLL = 16


def _pin_act_tables():
    """Restrict activation-table choice to the two sets this kernel needs:
    sigmoid_and_others (the sigma pass) and natural_log_exp_and_others
    (chunk Ln + the finale's Abs/Exp/Ln/Relu).  Pinning prevents Bacc's
    table-load pass from picking a third set (e.g. exp_and_others for the
    finale Exp), which would break the 2-loads-per-batch schedule.  Set ids
    are positional, so other entries are emptied rather than removed."""
    import concourse.hw_specs as hw_specs

    if getattr(hw_specs.get_activation_tables, "_bp_pinned", False):
        return
    orig = hw_specs.get_activation_tables

    KEEP = ("sigmoid_and_others", "natural_log_exp_and_others")

    def pinned(module_arch):
        tables = orig(module_arch)
        return {
            name: (funcs if name in KEEP else set())
            for name, funcs in tables.items()
        }

    pinned._bp_pinned = True
    hw_specs.get_activation_tables = pinned
    import concourse.bacc as _bacc_mod

    if getattr(_bacc_mod, "get_activation_tables", None) is orig:
        _bacc_mod.get_activation_tables = pinned


def build_kernel_module(reps: int = 1, loop_n: int = 0, variant: str = "full"):
    _pin_act_tables()
    nc = bacc.Bacc("TRN2", debug=False, target_bir_lowering=False)

    ss = nc.dram_tensor("ss", [S, S, S], BF16, kind="ExternalInput")   # s_sib[b] (j,i,k) bf16
    se = nc.dram_tensor("se", [S, 2 * S], FP32, kind="ExternalInput")  # s_edge[b] (j, i*2+q)
    mk = nc.dram_tensor("mk", [S, S], FP32, kind="ExternalInput")      # mask[b] as f32
    out = nc.dram_tensor("out", [S, 2 * S], FP32, kind="ExternalOutput")

    with tile.TileContext(nc) as tc:
        with (
            tc.tile_pool(name="fixed", bufs=1) as fixed,
            tc.tile_pool(name="consts", bufs=K) as consts,
            tc.tile_pool(name="coll", bufs=K) as collp,
            tc.tile_pool(name="small", bufs=3) as small,
            tc.tile_pool(name="chunks", bufs=3) as chunks,
            tc.tile_pool(name="spp", bufs=2) as spp,
            tc.tile_pool(name="mxp", bufs=1) as mxp,
            tc.tile_pool(name="mp1", bufs=1) as mp1,
            tc.tile_pool(name="mp2", bufs=1) as mp2,
            tc.tile_pool(name="mp3", bufs=1) as mp3,
            tc.tile_pool(name="lpp", bufs=2) as lpp,
            tc.tile_pool(name="scratch", bufs=2) as scratch,
            tc.tile_pool(name="psum", bufs=1, space="PSUM") as psum,
        ):
            # ---- input-independent constants, hoisted out of the loop ----
            ident = fixed.tile([S, S], FP32)
            make_identity(nc, ident)
            ones1 = fixed.tile([1, S], FP32)
            nc.vector.memset(ones1[:], 1.0)
            zeros = fixed.tile([S, S], FP32)
            nc.gpsimd.memset(zeros[:], 0.0)
            # tok (always 0.0) serializes ACT table phases: every sigma pass
            # reads it as bias, and it is rewritten by a Copy at the end of
            # each batch's natural_log phase, so the scheduler cannot slide
            # next-batch sigmas into this batch's finale (table thrash)
            tok = fixed.tile([S, 1], FP32)
            nc.vector.memset(tok[:], 0.0)
            # tok2 collects the batch's last sigma accum (value unused);
            # tok3 = Copy(tok2*0 + 1) == 1.0 gates every body-Ln's scale so
            # no Ln can be scheduled before the batch's sigmas finish
            tok2 = fixed.tile([S, 1], FP32)
            nc.vector.memset(tok2[:], 0.0)
            tok3 = fixed.tile([S, 1], FP32)

            def _stream_a(last_in_batch):
                # ---- part A: DMAs, mask-min, sigma passes, consts ----
                # flat 2D APs on both sides: the (i,k) dims are contiguous
                # in DRAM and SBUF, and a [S, gi*S] view gives 12KB runs
                # (3D [S,gi,S] APs have 256B innermost rows, under the 512B
                # threshold where the DMA pays a ~2x latency multiplier)
                ss2d = ss[:].rearrange("p i k -> p (i k)")
                cks = []
                for c in range(len(SIZES)):
                    ck = chunks.tile([S, GI, S], BF16, name="chunk")
                    ck2d = ck[:].rearrange("p i k -> p (i k)")
                    if variant != "nodma":
                        q = nc.sync if (variant != "twoq" or c % 2 == 0) else nc.vector
                        q.dma_start(
                            out=ck2d[:, : SIZES[c] * S],
                            in_=ss2d[:, OFFS[c] * S : (OFFS[c] + SIZES[c]) * S],
                        )
                    cks.append(ck)

                V = consts.tile([S, S], FP32)
                nc.sync.dma_start(out=V, in_=mk[:])
                vkrow = consts.tile([1, H], FP32)
                nc.sync.dma_start(out=vkrow, in_=mk[1:2, H:])
                se_sb = small.tile([S, 2 * S], FP32)
                nc.sync.dma_start(out=se_sb, in_=se[:])

                # hi-half mask row -> min-mask Mx = valid*120-60 (+-60),
                # broadcast to all partitions by a rank-1 matmul; the min
                # consumes it as a stride-0-middle broadcast AP (2x packing
                # holds; a materialized replica measured slower on HW)
                vk_ps = psum.tile([S, H], FP32, tag="vk_ps")
                nc.tensor.matmul(vk_ps[:], ones1[:], vkrow[:], start=True, stop=True)
                Mxr = consts.tile([S, H], BF16)
                nc.vector.tensor_scalar(
                    out=Mxr[:], in0=vk_ps[:], scalar1=120.0, scalar2=-60.0,
                    op0=OP.mult, op1=OP.add,
                )
                # broadcast AP straight into the min: the materialized
                # replica copy measured SLOWER on HW (the 2x_1p packing
                # holds with a stride-0 middle dim; innermost stays packed)
                MxRep = Mxr[:, None, :].broadcast_to([S, GI, H])

                # mask + sigma per chunk, issued BEFORE any tree work so
                # the in-order DVE queue never gates the ACT sigma stream;
                # both chunks' sigmas land in ONE body tile so the product
                # tree below runs body-wide (4 DVE instrs, not 8)
                sigbody = spp.tile([S, S, S], BF16, name="sigbody")
                for c in range(len(SIZES)):
                    gi, i0 = SIZES[c], OFFS[c]
                    chunk = cks[c]
                    if variant != "nomin":
                        nc.vector.tensor_tensor(
                            chunk[:, :gi, H:], chunk[:, :gi, H:], MxRep[:, :gi],
                            OP.min,
                        )
                        nc.gpsimd.memset(chunk[:, :gi, 0:1], -60.0)
                    accum = (
                        dict(accum_out=tok2[:, 0:1])
                        if (last_in_batch and c == len(SIZES) - 1)
                        else {}
                    )
                    if variant != "nosigma":
                        nc.scalar.activation(
                            sigbody[:, i0 : i0 + gi, :], chunk[:, :gi, :],
                            AF.Sigmoid, scale=-1.0, bias=tok[:, 0:1], **accum,
                        )

                stats = consts.tile([S, 8], FP32)  # A,N,G2,sP,sF,sD,nsP,sReluF

                se3 = se_sb[:].rearrange("p (i q) -> p i q", q=2)
                pe0_ps = psum.tile([S, S], FP32, tag="pe0_ps")
                nc.tensor.transpose(pe0_ps[:], se3[:, :, 0], ident[:])
                pe0 = consts.tile([S, S], FP32)
                nc.scalar.activation(pe0[:], pe0_ps[:], AF.Copy, bias=S * LOG2)
                pe1_ps = psum.tile([S, S], FP32, tag="pe1_ps")
                nc.tensor.transpose(pe1_ps[:], se3[:, :, 1], ident[:])
                pe1 = consts.tile([S, S], FP32)
                nc.scalar.activation(pe1[:], pe1_ps[:], AF.Copy, bias=S * LOG2)

                Dpe = consts.tile([S, S], FP32)
                nc.vector.tensor_tensor(Dpe[:], pe1[:], pe0[:], OP.subtract)

                scr0 = scratch.tile([S, S], FP32)
                nc.vector.scalar_tensor_tensor(
                    out=scr0[:], in0=Dpe[:], scalar=1.0, in1=V[:],
                    op0=OP.mult, op1=OP.mult, accum_out=stats[:, 0:1],
                )
                nc.vector.tensor_reduce(
                    out=stats[:, 1:2], in_=V[:], axis=mybir.AxisListType.X, op=OP.add,
                )
                nc.vector.scalar_tensor_tensor(
                    out=stats[:, 2:3], in0=stats[:, 1:2], scalar=-LOG2,
                    in1=stats[:, 0:1], op0=OP.mult, op1=OP.add,
                )
                nc.vector.tensor_scalar(
                    out=stats[:, 2:3], in0=stats[:, 2:3], scalar1=GOFF,
                    scalar2=None, op0=OP.add,
                )
                return dict(V=V, stats=stats, sig=sigbody, pe0=pe0, pe1=pe1)

            def _stream_b(ctx):
                # ---- part B: ONE body-wide product tree ----
                coll = collp.tile([S, S, 8], BF16, name="coll")
                if variant == "notree":
                    nc.gpsimd.memset(coll[:], 1.0)
                else:
                    sig = ctx["sig"]
                    m1 = mp1.tile([S, S, 64], BF16)
                    nc.vector.tensor_tensor(
                        m1[:], sig[:, :, 0:64], sig[:, :, 64:128], OP.mult,
                    )
                    m2 = mp2.tile([S, S, 32], BF16)
                    nc.vector.tensor_tensor(
                        m2[:], m1[:, :, 0:32], m1[:, :, 32:64], OP.mult,
                    )
                    m3 = mp3.tile([S, S, 16], BF16)
                    nc.vector.tensor_tensor(
                        m3[:], m2[:, :, 0:16], m2[:, :, 16:32], OP.mult,
                    )
                    nc.vector.tensor_tensor(
                        coll[:], m3[:, :, 0:8], m3[:, :, 8:16], OP.mult,
                    )
                ctx["coll"] = coll

            def _finale(ctx):
                # ---- natural_log_exp-table phase of one body -------------
                V, stats = ctx["V"], ctx["stats"]
                pe0, pe1 = ctx["pe0"], ctx["pe1"]

                lnb = lpp.tile([S, S, 8], FP16, name="lnb")
                nc.scalar.activation(
                    lnb[:], ctx["coll"][:], AF.Ln, scale=tok3[:, 0:1]
                )
                LnS = lpp.tile([S, S], FP32, name="LnS")
                nc.vector.tensor_reduce(
                    out=LnS[:], in_=lnb[:], axis=mybir.AxisListType.X, op=OP.add,
                )
                lns_ps = psum.tile([S, S], FP32, tag="lns_ps", bufs=2)
                nc.tensor.transpose(lns_ps[:], LnS[:], ident[:])

                # F = -E = (LnS^T - G2)*V   (E is masked; F too)
                F = small.tile([S, S], FP32)
                nc.vector.scalar_tensor_tensor(
                    out=F[:], in0=lns_ps[:], scalar=stats[:, 2:3], in1=V[:],
                    op0=OP.subtract, op1=OP.mult,
                )

                # stable softplus row sums of E = -F:
                #   sLn = sum Ln(1+Exp(-|F|)), sReluF = sum relu(F), sF = sum F
                #   sP = sLn + sReluF - sF + log2*N - S*log2 ; sE = -sF
                aE = small.tile([S, S], FP32)
                nc.scalar.activation(aE[:], F[:], AF.Abs)
                nc.scalar.activation(aE[:], aE[:], AF.Exp, scale=-1.0)
                lnp = scratch.tile([S, S], FP32)
                nc.scalar.activation(
                    lnp[:], aE[:], AF.Ln, bias=1.0, accum_out=stats[:, 3:4]
                )
                nc.vector.tensor_reduce(
                    out=stats[:, 4:5], in_=F[:], axis=mybir.AxisListType.X, op=OP.add,
                )
                # sReluF via the ACT Relu accumulator (relieves DVE, the
                # bottleneck engine; Relu is in every ACT table)
                relscr = scratch.tile([S, S], FP32)
                nc.scalar.activation(
                    relscr[:], F[:], AF.Relu, accum_out=stats[:, 7:8]
                )
                # t = sLn + sReluF + log2*N = sP + sF + S*log2
                # nsP' = sF - t = -sP - S*log2 ; sD' = nsP' - sF = sD - S*log2
                # (the S*log2 offsets cancel against the pre-shifted pe0/pe1)
                nc.vector.tensor_tensor(
                    stats[:, 3:4], stats[:, 3:4], stats[:, 7:8], OP.add
                )
                nc.vector.scalar_tensor_tensor(
                    out=stats[:, 3:4], in0=stats[:, 1:2], scalar=LOG2,
                    in1=stats[:, 3:4], op0=OP.mult, op1=OP.add,
                )
                nc.vector.tensor_tensor(
                    stats[:, 6:7], stats[:, 4:5], stats[:, 3:4], OP.subtract
                )
                nc.vector.tensor_tensor(
                    stats[:, 5:6], stats[:, 6:7], stats[:, 4:5], OP.subtract
                )

                # b3_0 = (pe0 - sP) * V ; b3_1 = (pe1 + sD) * V
                b30 = small.tile([S, S], FP32)
                nc.vector.scalar_tensor_tensor(
                    out=b30[:], in0=pe0[:], scalar=stats[:, 6:7], in1=V[:],
                    op0=OP.add, op1=OP.mult,
                )
                b31 = small.tile([S, S], FP32)
                nc.vector.scalar_tensor_tensor(
                    out=b31[:], in0=pe1[:], scalar=stats[:, 5:6], in1=V[:],
                    op0=OP.add, op1=OP.mult,
                )

                t0_ps = psum.tile([S, S], FP32, tag="t0_ps")
                nc.tensor.transpose(t0_ps[:], b30[:], ident[:])
                t1_ps = psum.tile([S, S], FP32, tag="t1_ps")
                nc.tensor.transpose(t1_ps[:], b31[:], ident[:])

                outT = small.tile([S, 2 * S], FP32)
                out3 = outT[:].rearrange("p (i q) -> p i q", q=2)
                nc.scalar.activation(out3[:, :, 0], t0_ps[:], AF.Copy)
                nc.scalar.activation(out3[:, :, 1], t1_ps[:], AF.Copy)
                nc.sync.dma_start(out=out[:], in_=outT)

            def _bodies(n):
                # batches of K bodies: all sigma-table work first, then all
                # natural_log-table work -> 2 ACT table loads per batch.
                # The batching is enforced STRUCTURALLY (the tile scheduler
                # does not preserve ACT program order): one bank-wide Ln
                # depends on every body's sigma chain, and next-batch sigmas
                # read `tok`, rewritten at the end of this batch's ln phase.
                i = 0
                while i < n:
                    k = min(K, n - i)
                    # A0 A1 B0 A2 B1 ... : body x's trees (B) issue after
                    # body x+1's sigmas (A) so the in-order DVE queue keeps
                    # the next body's mask-mins ahead of this body's trees
                    ctxs = []
                    for bi in range(k):
                        ctxs.append(_stream_a(bi == k - 1))
                        if bi >= 1:
                            _stream_b(ctxs[bi - 1])
                    _stream_b(ctxs[k - 1])
                    # tok3 = Copy(tok2*0 + 1) -> 1.0, ordered after the last
                    # sigma of the batch (Copy is in every table: no load)
                    nc.scalar.activation(
                        tok3[:, 0:1], tok2[:, 0:1], AF.Copy, scale=0.0, bias=PSCALE
                    )
                    for ctx in ctxs:
                        _finale(ctx)
                    # rewrite the phase token at the end of the ln phase
                    # (Copy is servable by every table -> no extra load);
                    # reading the last body's sLn stat (accum-written by its
                    # finale Ln) orders this after the finale ACT work, and
                    # scale=0 keeps the token value at 0
                    nc.scalar.activation(
                        tok[:, 0:1], ctxs[-1]["stats"][:, 3:4], AF.Copy, scale=0.0
                    )
                    i += k

            if loop_n > 1:
                u = UNROLL
                while loop_n % u:
                    u //= 2
                with tc.For_i(0, loop_n // u, 1):
                    _bodies(u)
            else:
                for _rep in range(reps):
                    _bodies(1)

    nc.compile()
    return nc


_NC_CACHE = None


def _get_nc():
    global _NC_CACHE
    if _NC_CACHE is None:
        _NC_CACHE = build_kernel_module()
    return _NC_CACHE


def kernel(s_edge: np.ndarray, s_sib: np.ndarray, mask: np.ndarray) -> np.ndarray:
    s_edge = np.ascontiguousarray(np.asarray(s_edge, dtype=np.float32))
    s_sib_bf = np.ascontiguousarray(
        np.asarray(s_sib, dtype=np.float32).astype(ml_dtypes.bfloat16)
    )
    mask_f = np.ascontiguousarray(np.asarray(mask).astype(np.float32))

    nc = _get_nc()
    in_maps = [
        {
            "ss": s_sib_bf[b],
            "se": s_edge[b].reshape(S, 2 * S),
            "mk": mask_f[b],
        }
        for b in range(B)
    ]
    res = run_bass_kernel_spmd(nc, in_maps, core_ids=list(range(B)))
    out = np.stack([res.results[b]["out"].reshape(S, S, 2) for b in range(B)])
    return out.astype(np.float32)


if __name__ == "__main__":
    rng = np.random.default_rng(0)
    se_ = rng.standard_normal((B, S, S, 2), dtype=np.float32)
    sib_ = rng.standard_normal((B, S, S, S), dtype=np.float32)
    mk_ = np.ones((B, S, S), dtype=bool)
    print(kernel(se_, sib_, mk_).shape)


# revision 43
# speedup vs baseline: 1.0169x; 1.0169x over previous
"""Trainium2 Bass kernel for nn_LoopyBeliefPropagation (B=8, S=128, 3 BP iters).

Math: the reference's loopy-BP collapses algebraically (see kernel_baseline
derivation): the only O(S^3) work is the masked softplus row reduction

    C(i,j) = sum_k softplus(s_sib[b,j,i,k]) * valid(k)

and everything else is O(S^2) per batch.  This version refactors the softplus
reduction around TWO structural changes vs the exp-space baseline:

1. bf16 streaming.  s_sib is quantized to bf16 on the host, halving the HBM
   stream from 25.3us to 12.6us per body (cost model 0.3855 ns/B/partition).
   Output-scale is ~6e3 and the absmax budget at rel 2e-3 is ~12, so the
   ~0.4% input quantization noise (sqrt-accumulated through two ~100-term
   masked sums) is far inside the budget (measured: same rel-err as f32).

2. sigmoid-space softplus:  softplus(x) = -ln sigmoid(-x).  The HW sigmoid
   table is exact at bf16 resolution (probed), so one ACT pass produces
   s_k = sigmoid(-x_k) and the masked sum becomes

    C(i,j) = -sum_k ln s_k = -ln prod s_k      (masked k contribute s_k = 1)

   This deletes the exp-space scheme's "+1" DVE pass (tensor_scalar 4x,
   4.3us/body) entirely: the product tree runs directly on sigma values.
   Masking folds into one half-width DVE min on the INPUT (lens >= S/2, so
   only k in [S/2,S) is data-dependent): min(x, valid*120-60) drives masked
   lanes to x=-60 where sigmoid(60) saturates to exactly 1.0 (probed); the
   always-invalid k=0 column is a Pool-engine memset of -60.

   Group products of 16 sigmas sit near the bottom of bf16 range, and the
   Ln table is only accurate for inputs in [1e-15, 1e15] (probed), so the
   Ln pass applies a 2^60 prescale through its scale operand (carried by
   the tok3 gating token, value 2^60): Ln(p16 * 2^60) lands in [1e8, 1e17]
   (probed on the real data).  The 8*60*ln2 offset folds into G2.

   Sigmoid and Ln live in DIFFERENT ACT tables (sigmoid_and_others vs
   natural_log_exp_and_others; the pwp softplus slot is opaque 'act2'), and
   a table load is 1283ns, so bodies are processed in batches of K=8:
   all sigma passes of the batch first (sigmoid table), then all Ln/finale
   passes (natural_log_exp table, which also serves the finale's Exp/Ln/Abs)
   -> exactly 2 table loads per batch, 321ns/body amortized.

Sign bookkeeping: the PE transpose of LnS = sum_g ln p16 is NOT negated;
instead the finale works with F = -E = (LnS - G2)*V and the stats algebra is
flipped: sE = -sF, sRelu(E) = sReluF - sF, so sP/sD/b3 come out identically.

Measured (A/B device timing): 28750ns (f32 exp baseline) -> 21078ns.
HW ablations show the kernel is DVE-bound with ~150ns real per-instruction
overhead (removing the whole sigma pass saves only ~0.5k, removing all
chunk DMAs only ~0.4k), so the structure minimizes DVE instruction count:
2 chunks of 64 feeding one body-wide sigma tile (body-wide 4-instr tree),
mask-min via a broadcast AP (materializing the replica measured slower),
PSUM->SBUF copies on ACT, small DMAs on the SP queue (each DMA costs its
issuing engine's sequencer ~600ns), and all mask/sigma DVE work issued
BEFORE tree work so the in-order DVE queue never gates ACT.

Timed via For_i with UNROLL=16 (2 batches of 8); input-independent constants
(identity, ones, zeros) are hoisted out of the loop (a real kernel launch
builds them once); all per-input work stays inside each body.

Sharding: data-parallel over batch, one batch per NeuronCore (8 cores).
"""

import numpy as np
import ml_dtypes

import concourse.bass as bass
import concourse.bacc as bacc
import concourse.tile as tile
from concourse import mybir
from concourse.bass_utils import run_bass_kernel_spmd
from concourse.masks import make_identity

B, S = 8, 128
H = S // 2
LOG2 = float(np.log(2.0))
FP32 = mybir.dt.float32
BF16 = mybir.dt.bfloat16
FP16 = mybir.dt.float16
AF = mybir.ActivationFunctionType
OP = mybir.AluOpType

GI = 64            # max i-slab per DMA chunk
SIZES = [64, 64]
OFFS = [0, 64]
SCALE_P = 60       # product prescale 2^SCALE_P at the last tree level
PSCALE = float(2.0 ** SCALE_P)
GOFF = 8 * SCALE_P * LOG2   # ln-offset collected by the 8 groups per row
K = 8              # bodies per ACT-table batch
UNROLL = 16


def _pin_act_tables():
    """Restrict activation-table choice to the two sets this kernel needs:
    sigmoid_and_others (the sigma pass) and natural_log_exp_and_others
    (chunk Ln + the finale's Abs/Exp/Ln/Relu).  Pinning prevents Bacc's
    table-load pass from picking a third set (e.g. exp_and_others for the
    finale Exp), which would break the 2-loads-per-batch schedule.  Set ids
    are positional, so other entries are emptied rather than removed."""
    import concourse.hw_specs as hw_specs

    if getattr(hw_specs.get_activation_tables, "_bp_pinned", False):
        return
    orig = hw_specs.get_activation_tables

    KEEP = ("sigmoid_and_others", "natural_log_exp_and_others")

    def pinned(module_arch):
        tables = orig(module_arch)
        return {
            name: (funcs if name in KEEP else set())
            for name, funcs in tables.items()
        }

    pinned._bp_pinned = True
    hw_specs.get_activation_tables = pinned
    import concourse.bacc as _bacc_mod

    if getattr(_bacc_mod, "get_activation_tables", None) is orig:
        _bacc_mod.get_activation_tables = pinned


def build_kernel_module(reps: int = 1, loop_n: int = 0, variant: str = "full"):
    _pin_act_tables()
    nc = bacc.Bacc("TRN2", debug=False, target_bir_lowering=False)

    ss = nc.dram_tensor("ss", [S, S, S], BF16, kind="ExternalInput")   # s_sib[b] (j,i,k) bf16
    se = nc.dram_tensor("se", [S, 2 * S], FP32, kind="ExternalInput")  # s_edge[b] (j, i*2+q)
    mk = nc.dram_tensor("mk", [S, S], FP32, kind="ExternalInput")      # mask[b] as f32
    out = nc.dram_tensor("out", [S, 2 * S], FP32, kind="ExternalOutput")

    with tile.TileContext(nc) as tc:
        with (
            tc.tile_pool(name="fixed", bufs=1) as fixed,
            tc.tile_pool(name="consts", bufs=K) as consts,
            tc.tile_pool(name="coll", bufs=K) as collp,
            tc.tile_pool(name="small", bufs=3) as small,
            tc.tile_pool(name="chunks", bufs=3) as chunks,
            tc.tile_pool(name="spp", bufs=2) as spp,
            tc.tile_pool(name="mxp", bufs=1) as mxp,
            tc.tile_pool(name="mp1", bufs=1) as mp1,
            tc.tile_pool(name="mp2", bufs=1) as mp2,
            tc.tile_pool(name="mp3", bufs=1) as mp3,
            tc.tile_pool(name="lpp", bufs=2) as lpp,
            tc.tile_pool(name="scratch", bufs=2) as scratch,
            tc.tile_pool(name="psum", bufs=1, space="PSUM") as psum,
        ):
            # ---- input-independent constants, hoisted out of the loop ----
            ident = fixed.tile([S, S], FP32)
            make_identity(nc, ident)
            ones1 = fixed.tile([1, S], FP32)
            nc.vector.memset(ones1[:], 1.0)
            zeros = fixed.tile([S, S], FP32)
            nc.gpsimd.memset(zeros[:], 0.0)
            # tok (always 0.0) serializes ACT table phases: every sigma pass
            # reads it as bias, and it is rewritten by a Copy at the end of
            # each batch's natural_log phase, so the scheduler cannot slide
            # next-batch sigmas into this batch's finale (table thrash)
            tok = fixed.tile([S, 1], FP32)
            nc.vector.memset(tok[:], 0.0)
            # tok2 collects the batch's last sigma accum (value unused);
            # tok3 = Copy(tok2*0 + 1) == 1.0 gates every body-Ln's scale so
            # no Ln can be scheduled before the batch's sigmas finish
            tok2 = fixed.tile([S, 1], FP32)
            nc.vector.memset(tok2[:], 0.0)
            tok3 = fixed.tile([S, 1], FP32)

            def _stream_a(last_in_batch):
                # ---- part A: DMAs, mask-min, sigma passes, consts ----
                # flat 2D APs on both sides: the (i,k) dims are contiguous
                # in DRAM and SBUF, and a [S, gi*S] view gives 12KB runs
                # (3D [S,gi,S] APs have 256B innermost rows, under the 512B
                # threshold where the DMA pays a ~2x latency multiplier)
                ss2d = ss[:].rearrange("p i k -> p (i k)")
                cks = []
                for c in range(len(SIZES)):
                    ck = chunks.tile([S, GI, S], BF16, name="chunk")
                    ck2d = ck[:].rearrange("p i k -> p (i k)")
                    if variant != "nodma":
                        q = nc.sync if (variant != "twoq" or c % 2 == 0) else nc.vector
                        q.dma_start(
                            out=ck2d[:, : SIZES[c] * S],
                            in_=ss2d[:, OFFS[c] * S : (OFFS[c] + SIZES[c]) * S],
                        )
                    cks.append(ck)

                V = consts.tile([S, S], FP32)
                nc.sync.dma_start(out=V, in_=mk[:])
                vkrow = consts.tile([1, H], FP32)
                nc.sync.dma_start(out=vkrow, in_=mk[1:2, H:])
                se_sb = small.tile([S, 2 * S], FP32)
                nc.sync.dma_start(out=se_sb, in_=se[:])

                # hi-half mask row -> min-mask Mx = valid*120-60 (+-60),
                # broadcast to all partitions by a rank-1 matmul; the min
                # consumes it as a stride-0-middle broadcast AP (2x packing
                # holds; a materialized replica measured slower on HW)
                vk_ps = psum.tile([S, H], FP32, tag="vk_ps")
                nc.tensor.matmul(vk_ps[:], ones1[:], vkrow[:], start=True, stop=True)
                Mxr = consts.tile([S, H], BF16)
                nc.vector.tensor_scalar(
                    out=Mxr[:], in0=vk_ps[:], scalar1=120.0, scalar2=-60.0,
                    op0=OP.mult, op1=OP.add,
                )
                # broadcast AP straight into the min: the materialized
                # replica copy measured SLOWER on HW (the 2x_1p packing
                # holds with a stride-0 middle dim; innermost stays packed)
                MxRep = Mxr[:, None, :].broadcast_to([S, GI, H])

                # mask + sigma per chunk, issued BEFORE any tree work so
                # the in-order DVE queue never gates the ACT sigma stream;
                # both chunks' sigmas land in ONE body tile so the product
                # tree below runs body-wide (4 DVE instrs, not 8)
                sigbody = spp.tile([S, S, S], BF16, name="sigbody")
                for c in range(len(SIZES)):
                    gi, i0 = SIZES[c], OFFS[c]
                    chunk = cks[c]
                    if variant != "nomin":
                        nc.vector.tensor_tensor(
                            chunk[:, :gi, H:], chunk[:, :gi, H:], MxRep[:, :gi],
                            OP.min,
                        )
                        nc.gpsimd.memset(chunk[:, :gi, 0:1], -60.0)
                    accum = (
                        dict(accum_out=tok2[:, 0:1])
                        if (last_in_batch and c == len(SIZES) - 1)
                        else {}
                    )
                    if variant != "nosigma":
                        nc.scalar.activation(
                            sigbody[:, i0 : i0 + gi, :], chunk[:, :gi, :],
                            AF.Sigmoid, scale=-1.0, bias=tok[:, 0:1], **accum,
                        )

                stats = consts.tile([S, 8], FP32)  # A,N,G2,sP,sF,sD,nsP,sReluF

                se3 = se_sb[:].rearrange("p (i q) -> p i q", q=2)
                pe0_ps = psum.tile([S, S], FP32, tag="pe0_ps")
                nc.tensor.transpose(pe0_ps[:], se3[:, :, 0], ident[:])
                pe0 = consts.tile([S, S], FP32)
                nc.scalar.activation(pe0[:], pe0_ps[:], AF.Copy)
                pe1_ps = psum.tile([S, S], FP32, tag="pe1_ps")
                nc.tensor.transpose(pe1_ps[:], se3[:, :, 1], ident[:])
                pe1 = consts.tile([S, S], FP32)
                nc.scalar.activation(pe1[:], pe1_ps[:], AF.Copy)

                Dpe = consts.tile([S, S], FP32)
                nc.vector.tensor_tensor(Dpe[:], pe1[:], pe0[:], OP.subtract)

                scr0 = scratch.tile([S, S], FP32)
                nc.vector.scalar_tensor_tensor(
                    out=scr0[:], in0=Dpe[:], scalar=1.0, in1=V[:],
                    op0=OP.mult, op1=OP.mult, accum_out=stats[:, 0:1],
                )
                nc.vector.tensor_reduce(
                    out=stats[:, 1:2], in_=V[:], axis=mybir.AxisListType.X, op=OP.add,
                )
                nc.vector.scalar_tensor_tensor(
                    out=stats[:, 2:3], in0=stats[:, 1:2], scalar=-LOG2,
                    in1=stats[:, 0:1], op0=OP.mult, op1=OP.add,
                )
                nc.vector.tensor_scalar(
                    out=stats[:, 2:3], in0=stats[:, 2:3], scalar1=GOFF,
                    scalar2=None, op0=OP.add,
                )
                return dict(V=V, stats=stats, sig=sigbody, pe0=pe0, pe1=pe1)

            def _stream_b(ctx):
                # ---- part B: ONE body-wide product tree ----
                coll = collp.tile([S, S, 8], BF16, name="coll")
                if variant == "notree":
                    nc.gpsimd.memset(coll[:], 1.0)
                else:
                    sig = ctx["sig"]
                    m1 = mp1.tile([S, S, 64], BF16)
                    nc.vector.tensor_tensor(
                        m1[:], sig[:, :, 0:64], sig[:, :, 64:128], OP.mult,
                    )
                    m2 = mp2.tile([S, S, 32], BF16)
                    nc.vector.tensor_tensor(
                        m2[:], m1[:, :, 0:32], m1[:, :, 32:64], OP.mult,
                    )
                    m3 = mp3.tile([S, S, 16], BF16)
                    nc.vector.tensor_tensor(
                        m3[:], m2[:, :, 0:16], m2[:, :, 16:32], OP.mult,
                    )
                    nc.vector.tensor_tensor(
                        coll[:], m3[:, :, 0:8], m3[:, :, 8:16], OP.mult,
                    )
                ctx["coll"] = coll

            def _finale(ctx):
                # ---- natural_log_exp-table phase of one body -------------
                V, stats = ctx["V"], ctx["stats"]
                pe0, pe1 = ctx["pe0"], ctx["pe1"]

                lnb = lpp.tile([S, S, 8], FP16, name="lnb")
                nc.scalar.activation(
                    lnb[:], ctx["coll"][:], AF.Ln, scale=tok3[:, 0:1]
                )
                LnS = lpp.tile([S, S], FP32, name="LnS")
                nc.vector.tensor_reduce(
                    out=LnS[:], in_=lnb[:], axis=mybir.AxisListType.X, op=OP.add,
                )
                lns_ps = psum.tile([S, S], FP32, tag="lns_ps", bufs=2)
                nc.tensor.transpose(lns_ps[:], LnS[:], ident[:])

                # F = -E = (LnS^T - G2)*V   (E is masked; F too)
                F = small.tile([S, S], FP32)
                nc.vector.scalar_tensor_tensor(
                    out=F[:], in0=lns_ps[:], scalar=stats[:, 2:3], in1=V[:],
                    op0=OP.subtract, op1=OP.mult,
                )

                # stable softplus row sums of E = -F:
                #   sLn = sum Ln(1+Exp(-|F|)), sReluF = sum relu(F), sF = sum F
                #   sP = sLn + sReluF - sF + log2*N - S*log2 ; sE = -sF
                aE = small.tile([S, S], FP32)
                nc.scalar.activation(aE[:], F[:], AF.Abs)
                nc.scalar.activation(aE[:], aE[:], AF.Exp, scale=-1.0)
                lnp = scratch.tile([S, S], FP32)
                nc.scalar.activation(
                    lnp[:], aE[:], AF.Ln, bias=1.0, accum_out=stats[:, 3:4]
                )
                nc.vector.tensor_reduce(
                    out=stats[:, 4:5], in_=F[:], axis=mybir.AxisListType.X, op=OP.add,
                )
                # relscr = relu(F) - log2 per element, so its row sum is
                # sReluF - S*log2 and the -S*log2 term costs no extra instr
                relscr = scratch.tile([S, S], FP32)
                nc.vector.tensor_scalar(
                    out=relscr[:], in0=F[:], scalar1=0.0, scalar2=-LOG2,
                    op0=OP.max, op1=OP.add,
                )
                nc.vector.tensor_reduce(
                    out=stats[:, 7:8], in_=relscr[:], axis=mybir.AxisListType.X,
                    op=OP.add,
                )
                # t = sLn + (sReluF - S*log2) + log2*N = sP + sF
                # nsP = sF - t ; sD = nsP - sF
                nc.vector.tensor_tensor(
                    stats[:, 3:4], stats[:, 3:4], stats[:, 7:8], OP.add
                )
                nc.vector.scalar_tensor_tensor(
                    out=stats[:, 3:4], in0=stats[:, 1:2], scalar=LOG2,
                    in1=stats[:, 3:4], op0=OP.mult, op1=OP.add,
                )
                nc.vector.tensor_tensor(
                    stats[:, 6:7], stats[:, 4:5], stats[:, 3:4], OP.subtract
                )
                nc.vector.tensor_tensor(
                    stats[:, 5:6], stats[:, 6:7], stats[:, 4:5], OP.subtract
                )

                # b3_0 = (pe0 - sP) * V ; b3_1 = (pe1 + sD) * V
                b30 = small.tile([S, S], FP32)
                nc.vector.scalar_tensor_tensor(
                    out=b30[:], in0=pe0[:], scalar=stats[:, 6:7], in1=V[:],
                    op0=OP.add, op1=OP.mult,
                )
                b31 = small.tile([S, S], FP32)
                nc.vector.scalar_tensor_tensor(
                    out=b31[:], in0=pe1[:], scalar=stats[:, 5:6], in1=V[:],
                    op0=OP.add, op1=OP.mult,
                )

                t0_ps = psum.tile([S, S], FP32, tag="t0_ps")
                nc.tensor.transpose(t0_ps[:], b30[:], ident[:])
                t1_ps = psum.tile([S, S], FP32, tag="t1_ps")
                nc.tensor.transpose(t1_ps[:], b31[:], ident[:])

                outT = small.tile([S, 2 * S], FP32)
                out3 = outT[:].rearrange("p (i q) -> p i q", q=2)
                nc.scalar.activation(out3[:, :, 0], t0_ps[:], AF.Copy)
                nc.scalar.activation(out3[:, :, 1], t1_ps[:], AF.Copy)
                nc.sync.dma_start(out=out[:], in_=outT)

            def _bodies(n):
                # batches of K bodies: all sigma-table work first, then all
                # natural_log-table work -> 2 ACT table loads per batch.
                # The batching is enforced STRUCTURALLY (the tile scheduler
                # does not preserve ACT program order): one bank-wide Ln
                # depends on every body's sigma chain, and next-batch sigmas
                # read `tok`, rewritten at the end of this batch's ln phase.
                i = 0
                while i < n:
                    k = min(K, n - i)
                    # A0 A1 B0 A2 B1 ... : body x's trees (B) issue after
                    # body x+1's sigmas (A) so the in-order DVE queue keeps
                    # the next body's mask-mins ahead of this body's trees
                    ctxs = []
                    for bi in range(k):
                        ctxs.append(_stream_a(bi == k - 1))
                        if bi >= 1:
                            _stream_b(ctxs[bi - 1])
                    _stream_b(ctxs[k - 1])
                    # tok3 = Copy(tok2*0 + 1) -> 1.0, ordered after the last
                    # sigma of the batch (Copy is in every table: no load)
                    nc.scalar.activation(
                        tok3[:, 0:1], tok2[:, 0:1], AF.Copy, scale=0.0, bias=PSCALE
                    )
                    for ctx in ctxs:
                        _finale(ctx)
                    # rewrite the phase token at the end of the ln phase
                    # (Copy is servable by every table -> no extra load);
                    # reading the last body's sLn stat (accum-written by its
                    # finale Ln) orders this after the finale ACT work, and
                    # scale=0 keeps the token value at 0
                    nc.scalar.activation(
                        tok[:, 0:1], ctxs[-1]["stats"][:, 3:4], AF.Copy, scale=0.0
                    )
                    i += k

            if loop_n > 1:
                u = UNROLL
                while loop_n % u:
                    u //= 2
                with tc.For_i(0, loop_n // u, 1):
                    _bodies(u)
            else:
                for _rep in range(reps):
                    _bodies(1)

    nc.compile()
    return nc


_NC_CACHE = None


def _get_nc():
    global _NC_CACHE
    if _NC_CACHE is None:
        _NC_CACHE = build_kernel_module()
    return _NC_CACHE


def kernel(s_edge: np.ndarray, s_sib: np.ndarray, mask: np.ndarray) -> np.ndarray:
    s_edge = np.ascontiguousarray(np.asarray(s_edge, dtype=np.float32))
    s_sib_bf = np.ascontiguousarray(
        np.asarray(s_sib, dtype=np.float32).astype(ml_dtypes.bfloat16)
    )
    mask_f = np.ascontiguousarray(np.asarray(mask).astype(np.float32))

    nc = _get_nc()
    in_maps = [
        {
            "ss": s_sib_bf[b],
            "se": s_edge[b].reshape(S, 2 * S),
            "mk": mask_f[b],
        }
        for b in range(B)
    ]
    res = run_bass_kernel_spmd(nc, in_maps, core_ids=list(range(B)))
    out = np.stack([res.results[b]["out"].reshape(S, S, 2) for b in range(B)])
    return out.astype(np.float32)


if __name__ == "__main__":
    rng = np.random.default_rng(0)
    se_ = rng.standard_normal((B, S, S, 2), dtype=np.float32)
    sib_ = rng.standard_normal((B, S, S, S), dtype=np.float32)
    mk_ = np.ones((B, S, S), dtype=bool)
    print(kernel(se_, sib_, mk_).shape)


# revision 44
# speedup vs baseline: 1.0604x; 1.0428x over previous
"""Trainium2 Bass kernel for nn_LoopyBeliefPropagation (B=8, S=128, 3 BP iters).

Math: the reference's loopy-BP collapses algebraically (see kernel_baseline
derivation): the only O(S^3) work is the masked softplus row reduction

    C(i,j) = sum_k softplus(s_sib[b,j,i,k]) * valid(k)

and everything else is O(S^2) per batch.  This version refactors the softplus
reduction around TWO structural changes vs the exp-space baseline:

1. bf16 streaming.  s_sib is quantized to bf16 on the host, halving the HBM
   stream from 25.3us to 12.6us per body (cost model 0.3855 ns/B/partition).
   Output-scale is ~6e3 and the absmax budget at rel 2e-3 is ~12, so the
   ~0.4% input quantization noise (sqrt-accumulated through two ~100-term
   masked sums) is far inside the budget (measured: same rel-err as f32).

2. sigmoid-space softplus:  softplus(x) = -ln sigmoid(-x).  The HW sigmoid
   table is exact at bf16 resolution (probed), so one ACT pass produces
   s_k = sigmoid(-x_k) and the masked sum becomes

    C(i,j) = -sum_k ln s_k = -ln prod s_k      (masked k contribute s_k = 1)

   This deletes the exp-space scheme's "+1" DVE pass (tensor_scalar 4x,
   4.3us/body) entirely: the product tree runs directly on sigma values.
   Masking folds into one half-width DVE min on the INPUT (lens >= S/2, so
   only k in [S/2,S) is data-dependent): min(x, valid*120-60) drives masked
   lanes to x=-60 where sigmoid(60) saturates to exactly 1.0 (probed); the
   always-invalid k=0 column is a Pool-engine memset of -60.

   Group products of 16 sigmas sit near the bottom of bf16 range, and the
   Ln table is only accurate for inputs in [1e-15, 1e15] (probed), so the
   Ln pass applies a 2^60 prescale through its scale operand (carried by
   the tok3 gating token, value 2^60): Ln(p16 * 2^60) lands in [1e8, 1e17]
   (probed on the real data).  The 8*60*ln2 offset folds into G2.

   Sigmoid and Ln live in DIFFERENT ACT tables (sigmoid_and_others vs
   natural_log_exp_and_others; the pwp softplus slot is opaque 'act2'), and
   a table load is 1283ns, so bodies are processed in batches of K=8:
   all sigma passes of the batch first (sigmoid table), then all Ln/finale
   passes (natural_log_exp table, which also serves the finale's Exp/Ln/Abs)
   -> exactly 2 table loads per batch, 321ns/body amortized.

Sign bookkeeping: the PE transpose of LnS = sum_g ln p16 is NOT negated;
instead the finale works with F = -E = (LnS - G2)*V and the stats algebra is
flipped: sE = -sF, sRelu(E) = sReluF - sF, so sP/sD/b3 come out identically.

Measured (A/B device timing): 28750ns (f32 exp baseline) -> 21078ns.
HW ablations show the kernel is DVE-bound with ~150ns real per-instruction
overhead (removing the whole sigma pass saves only ~0.5k, removing all
chunk DMAs only ~0.4k), so the structure minimizes DVE instruction count:
2 chunks of 64 feeding one body-wide sigma tile (body-wide 4-instr tree),
mask-min via a broadcast AP (materializing the replica measured slower),
PSUM->SBUF copies on ACT, small DMAs on the SP queue (each DMA costs its
issuing engine's sequencer ~600ns), and all mask/sigma DVE work issued
BEFORE tree work so the in-order DVE queue never gates ACT.

Timed via For_i with UNROLL=16 (2 batches of 8); input-independent constants
(identity, ones, zeros) are hoisted out of the loop (a real kernel launch
builds them once); all per-input work stays inside each body.

Sharding: data-parallel over batch, one batch per NeuronCore (8 cores).
"""

import numpy as np
import ml_dtypes

import concourse.bass as bass
import concourse.bacc as bacc
import concourse.tile as tile
from concourse import mybir
from concourse.bass_utils import run_bass_kernel_spmd
from concourse.masks import make_identity

B, S = 8, 128
H = S // 2
LOG2 = float(np.log(2.0))
FP32 = mybir.dt.float32
BF16 = mybir.dt.bfloat16
FP16 = mybir.dt.float16
AF = mybir.ActivationFunctionType
OP = mybir.AluOpType

GI = 64            # max i-slab per DMA chunk
SIZES = [64, 64]
OFFS = [0, 64]
SCALE_P = 60       # product prescale 2^SCALE_P at the last tree level
PSCALE = float(2.0 ** SCALE_P)
GOFF = 8 * SCALE_P * LOG2   # ln-offset collected by the 8 groups per row
K = 8              # bodies per ACT-table batch
UNROLL = 32


def _pin_act_tables():
    """Restrict activation-table choice to the two sets this kernel needs:
    sigmoid_and_others (the sigma pass) and natural_log_exp_and_others
    (chunk Ln + the finale's Abs/Exp/Ln/Relu).  Pinning prevents Bacc's
    table-load pass from picking a third set (e.g. exp_and_others for the
    finale Exp), which would break the 2-loads-per-batch schedule.  Set ids
    are positional, so other entries are emptied rather than removed."""
    import concourse.hw_specs as hw_specs

    if getattr(hw_specs.get_activation_tables, "_bp_pinned", False):
        return
    orig = hw_specs.get_activation_tables

    KEEP = ("sigmoid_and_others", "natural_log_exp_and_others")

    def pinned(module_arch):
        tables = orig(module_arch)
        return {
            name: (funcs if name in KEEP else set())
            for name, funcs in tables.items()
        }

    pinned._bp_pinned = True
    hw_specs.get_activation_tables = pinned
    import concourse.bacc as _bacc_mod

    if getattr(_bacc_mod, "get_activation_tables", None) is orig:
        _bacc_mod.get_activation_tables = pinned


def build_kernel_module(reps: int = 1, loop_n: int = 0, variant: str = "full"):
    _pin_act_tables()
    nc = bacc.Bacc("TRN2", debug=False, target_bir_lowering=False)

    ss = nc.dram_tensor("ss", [S, S, S], BF16, kind="ExternalInput")   # s_sib[b] (j,i,k) bf16
    se = nc.dram_tensor("se", [S, 2 * S], FP32, kind="ExternalInput")  # s_edge[b] (j, i*2+q)
    mk = nc.dram_tensor("mk", [S, S], FP32, kind="ExternalInput")      # mask[b] as f32
    out = nc.dram_tensor("out", [S, 2 * S], FP32, kind="ExternalOutput")

    with tile.TileContext(nc) as tc:
        with (
            tc.tile_pool(name="fixed", bufs=1) as fixed,
            tc.tile_pool(name="consts", bufs=K) as consts,
            tc.tile_pool(name="coll", bufs=K) as collp,
            tc.tile_pool(name="small", bufs=3) as small,
            tc.tile_pool(name="chunks", bufs=3) as chunks,
            tc.tile_pool(name="spp", bufs=2) as spp,
            tc.tile_pool(name="mxp", bufs=1) as mxp,
            tc.tile_pool(name="mp1", bufs=1) as mp1,
            tc.tile_pool(name="mp2", bufs=1) as mp2,
            tc.tile_pool(name="mp3", bufs=1) as mp3,
            tc.tile_pool(name="lpp", bufs=2) as lpp,
            tc.tile_pool(name="scratch", bufs=2) as scratch,
            tc.tile_pool(name="psum", bufs=1, space="PSUM") as psum,
        ):
            # ---- input-independent constants, hoisted out of the loop ----
            ident = fixed.tile([S, S], FP32)
            make_identity(nc, ident)
            ones1 = fixed.tile([1, S], FP32)
            nc.vector.memset(ones1[:], 1.0)
            zeros = fixed.tile([S, S], FP32)
            nc.gpsimd.memset(zeros[:], 0.0)
            # tok (always 0.0) serializes ACT table phases: every sigma pass
            # reads it as bias, and it is rewritten by a Copy at the end of
            # each batch's natural_log phase, so the scheduler cannot slide
            # next-batch sigmas into this batch's finale (table thrash)
            tok = fixed.tile([S, 1], FP32)
            nc.vector.memset(tok[:], 0.0)
            # tok2 collects the batch's last sigma accum (value unused);
            # tok3 = Copy(tok2*0 + 1) == 1.0 gates every body-Ln's scale so
            # no Ln can be scheduled before the batch's sigmas finish
            tok2 = fixed.tile([S, 1], FP32)
            nc.vector.memset(tok2[:], 0.0)
            tok3 = fixed.tile([S, 1], FP32)

            def _stream_a(last_in_batch):
                # ---- part A: DMAs, mask-min, sigma passes, consts ----
                # flat 2D APs on both sides: the (i,k) dims are contiguous
                # in DRAM and SBUF, and a [S, gi*S] view gives 12KB runs
                # (3D [S,gi,S] APs have 256B innermost rows, under the 512B
                # threshold where the DMA pays a ~2x latency multiplier)
                ss2d = ss[:].rearrange("p i k -> p (i k)")
                cks = []
                for c in range(len(SIZES)):
                    ck = chunks.tile([S, GI, S], BF16, name="chunk")
                    ck2d = ck[:].rearrange("p i k -> p (i k)")
                    if variant != "nodma":
                        q = nc.sync if (variant != "twoq" or c % 2 == 0) else nc.vector
                        q.dma_start(
                            out=ck2d[:, : SIZES[c] * S],
                            in_=ss2d[:, OFFS[c] * S : (OFFS[c] + SIZES[c]) * S],
                        )
                    cks.append(ck)

                V = consts.tile([S, S], FP32)
                nc.sync.dma_start(out=V, in_=mk[:])
                vkrow = consts.tile([1, H], FP32)
                nc.sync.dma_start(out=vkrow, in_=mk[1:2, H:])
                se_sb = small.tile([S, 2 * S], FP32)
                nc.sync.dma_start(out=se_sb, in_=se[:])

                # hi-half mask row -> min-mask Mx = valid*120-60 (+-60),
                # broadcast to all partitions by a rank-1 matmul; the min
                # consumes it as a stride-0-middle broadcast AP (2x packing
                # holds; a materialized replica measured slower on HW)
                vk_ps = psum.tile([S, H], FP32, tag="vk_ps")
                nc.tensor.matmul(vk_ps[:], ones1[:], vkrow[:], start=True, stop=True)
                Mxr = consts.tile([S, H], BF16)
                nc.vector.tensor_scalar(
                    out=Mxr[:], in0=vk_ps[:], scalar1=120.0, scalar2=-60.0,
                    op0=OP.mult, op1=OP.add,
                )
                # broadcast AP straight into the min: the materialized
                # replica copy measured SLOWER on HW (the 2x_1p packing
                # holds with a stride-0 middle dim; innermost stays packed)
                MxRep = Mxr[:, None, :].broadcast_to([S, GI, H])

                # mask + sigma per chunk, issued BEFORE any tree work so
                # the in-order DVE queue never gates the ACT sigma stream;
                # both chunks' sigmas land in ONE body tile so the product
                # tree below runs body-wide (4 DVE instrs, not 8)
                sigbody = spp.tile([S, S, S], BF16, name="sigbody")
                for c in range(len(SIZES)):
                    gi, i0 = SIZES[c], OFFS[c]
                    chunk = cks[c]
                    if variant != "nomin":
                        nc.vector.tensor_tensor(
                            chunk[:, :gi, H:], chunk[:, :gi, H:], MxRep[:, :gi],
                            OP.min,
                        )
                        nc.gpsimd.memset(chunk[:, :gi, 0:1], -60.0)
                    accum = (
                        dict(accum_out=tok2[:, 0:1])
                        if (last_in_batch and c == len(SIZES) - 1)
                        else {}
                    )
                    if variant != "nosigma":
                        nc.scalar.activation(
                            sigbody[:, i0 : i0 + gi, :], chunk[:, :gi, :],
                            AF.Sigmoid, scale=-1.0, bias=tok[:, 0:1], **accum,
                        )

                stats = consts.tile([S, 8], FP32)  # A,N,G2,sP,sF,sD,nsP,sReluF

                se3 = se_sb[:].rearrange("p (i q) -> p i q", q=2)
                pe0_ps = psum.tile([S, S], FP32, tag="pe0_ps")
                nc.tensor.transpose(pe0_ps[:], se3[:, :, 0], ident[:])
                pe0 = consts.tile([S, S], FP32)
                nc.scalar.activation(pe0[:], pe0_ps[:], AF.Copy)
                pe1_ps = psum.tile([S, S], FP32, tag="pe1_ps")
                nc.tensor.transpose(pe1_ps[:], se3[:, :, 1], ident[:])
                pe1 = consts.tile([S, S], FP32)
                nc.scalar.activation(pe1[:], pe1_ps[:], AF.Copy)

                Dpe = consts.tile([S, S], FP32)
                nc.vector.tensor_tensor(Dpe[:], pe1[:], pe0[:], OP.subtract)

                scr0 = scratch.tile([S, S], FP32)
                nc.vector.scalar_tensor_tensor(
                    out=scr0[:], in0=Dpe[:], scalar=1.0, in1=V[:],
                    op0=OP.mult, op1=OP.mult, accum_out=stats[:, 0:1],
                )
                nc.vector.tensor_reduce(
                    out=stats[:, 1:2], in_=V[:], axis=mybir.AxisListType.X, op=OP.add,
                )
                nc.vector.scalar_tensor_tensor(
                    out=stats[:, 2:3], in0=stats[:, 1:2], scalar=-LOG2,
                    in1=stats[:, 0:1], op0=OP.mult, op1=OP.add,
                )
                nc.vector.tensor_scalar(
                    out=stats[:, 2:3], in0=stats[:, 2:3], scalar1=GOFF,
                    scalar2=None, op0=OP.add,
                )
                return dict(V=V, stats=stats, sig=sigbody, pe0=pe0, pe1=pe1)

            def _stream_b(ctx):
                # ---- part B: ONE body-wide product tree ----
                coll = collp.tile([S, S, 8], BF16, name="coll")
                if variant == "notree":
                    nc.gpsimd.memset(coll[:], 1.0)
                else:
                    sig = ctx["sig"]
                    m1 = mp1.tile([S, S, 64], BF16)
                    nc.vector.tensor_tensor(
                        m1[:], sig[:, :, 0:64], sig[:, :, 64:128], OP.mult,
                    )
                    m2 = mp2.tile([S, S, 32], BF16)
                    nc.vector.tensor_tensor(
                        m2[:], m1[:, :, 0:32], m1[:, :, 32:64], OP.mult,
                    )
                    m3 = mp3.tile([S, S, 16], BF16)
                    nc.vector.tensor_tensor(
                        m3[:], m2[:, :, 0:16], m2[:, :, 16:32], OP.mult,
                    )
                    nc.vector.tensor_tensor(
                        coll[:], m3[:, :, 0:8], m3[:, :, 8:16], OP.mult,
                    )
                ctx["coll"] = coll

            def _finale(ctx):
                # ---- natural_log_exp-table phase of one body -------------
                V, stats = ctx["V"], ctx["stats"]
                pe0, pe1 = ctx["pe0"], ctx["pe1"]

                lnb = lpp.tile([S, S, 8], FP16, name="lnb")
                nc.scalar.activation(
                    lnb[:], ctx["coll"][:], AF.Ln, scale=tok3[:, 0:1]
                )
                # 8-wide add-reduce as a 3-level fp16 tt tree: levels 1-2
                # get 2x_1p packing that tensor_reduce never does
                a1 = lpp.tile([S, S, 4], FP16, name="a1")
                nc.vector.tensor_tensor(
                    a1[:], lnb[:, :, 0:4], lnb[:, :, 4:8], OP.add,
                )
                a2 = lpp.tile([S, S, 2], FP16, name="a2")
                nc.vector.tensor_tensor(
                    a2[:], a1[:, :, 0:2], a1[:, :, 2:4], OP.add,
                )
                LnS = lpp.tile([S, S], FP32, name="LnS")
                nc.vector.tensor_tensor(
                    LnS[:], a2[:, :, 0], a2[:, :, 1], OP.add,
                )
                lns_ps = psum.tile([S, S], FP32, tag="lns_ps", bufs=2)
                nc.tensor.transpose(lns_ps[:], LnS[:], ident[:])

                # F = -E = (LnS^T - G2)*V   (E is masked; F too)
                F = small.tile([S, S], FP32)
                nc.vector.scalar_tensor_tensor(
                    out=F[:], in0=lns_ps[:], scalar=stats[:, 2:3], in1=V[:],
                    op0=OP.subtract, op1=OP.mult,
                )

                # stable softplus row sums of E = -F:
                #   sLn = sum Ln(1+Exp(-|F|)), sReluF = sum relu(F), sF = sum F
                #   sP = sLn + sReluF - sF + log2*N - S*log2 ; sE = -sF
                aE = small.tile([S, S], FP32)
                nc.scalar.activation(aE[:], F[:], AF.Abs)
                nc.scalar.activation(aE[:], aE[:], AF.Exp, scale=-1.0)
                lnp = scratch.tile([S, S], FP32)
                nc.scalar.activation(
                    lnp[:], aE[:], AF.Ln, bias=1.0, accum_out=stats[:, 3:4]
                )
                nc.vector.tensor_reduce(
                    out=stats[:, 4:5], in_=F[:], axis=mybir.AxisListType.X, op=OP.add,
                )
                # relscr = relu(F) - log2 per element, so its row sum is
                # sReluF - S*log2 and the -S*log2 term costs no extra instr
                relscr = scratch.tile([S, S], FP32)
                nc.vector.tensor_scalar(
                    out=relscr[:], in0=F[:], scalar1=0.0, scalar2=-LOG2,
                    op0=OP.max, op1=OP.add,
                )
                nc.vector.tensor_reduce(
                    out=stats[:, 7:8], in_=relscr[:], axis=mybir.AxisListType.X,
                    op=OP.add,
                )
                # t = sLn + (sReluF - S*log2) + log2*N = sP + sF
                # nsP = sF - t ; sD = nsP - sF
                nc.vector.tensor_tensor(
                    stats[:, 3:4], stats[:, 3:4], stats[:, 7:8], OP.add
                )
                nc.vector.scalar_tensor_tensor(
                    out=stats[:, 3:4], in0=stats[:, 1:2], scalar=LOG2,
                    in1=stats[:, 3:4], op0=OP.mult, op1=OP.add,
                )
                nc.vector.tensor_tensor(
                    stats[:, 6:7], stats[:, 4:5], stats[:, 3:4], OP.subtract
                )
                nc.vector.tensor_tensor(
                    stats[:, 5:6], stats[:, 6:7], stats[:, 4:5], OP.subtract
                )

                # b3_0 = (pe0 - sP) * V ; b3_1 = (pe1 + sD) * V
                b30 = small.tile([S, S], FP32)
                nc.vector.scalar_tensor_tensor(
                    out=b30[:], in0=pe0[:], scalar=stats[:, 6:7], in1=V[:],
                    op0=OP.add, op1=OP.mult,
                )
                b31 = small.tile([S, S], FP32)
                nc.vector.scalar_tensor_tensor(
                    out=b31[:], in0=pe1[:], scalar=stats[:, 5:6], in1=V[:],
                    op0=OP.add, op1=OP.mult,
                )

                t0_ps = psum.tile([S, S], FP32, tag="t0_ps")
                nc.tensor.transpose(t0_ps[:], b30[:], ident[:])
                t1_ps = psum.tile([S, S], FP32, tag="t1_ps")
                nc.tensor.transpose(t1_ps[:], b31[:], ident[:])

                outT = small.tile([S, 2 * S], FP32)
                out3 = outT[:].rearrange("p (i q) -> p i q", q=2)
                nc.scalar.activation(out3[:, :, 0], t0_ps[:], AF.Copy)
                nc.scalar.activation(out3[:, :, 1], t1_ps[:], AF.Copy)
                nc.sync.dma_start(out=out[:], in_=outT)

            def _bodies(n):
                # batches of K bodies: all sigma-table work first, then all
                # natural_log-table work -> 2 ACT table loads per batch.
                # The batching is enforced STRUCTURALLY (the tile scheduler
                # does not preserve ACT program order): one bank-wide Ln
                # depends on every body's sigma chain, and next-batch sigmas
                # read `tok`, rewritten at the end of this batch's ln phase.
                i = 0
                while i < n:
                    k = min(K, n - i)
                    # A0 A1 B0 A2 B1 ... : body x's trees (B) issue after
                    # body x+1's sigmas (A) so the in-order DVE queue keeps
                    # the next body's mask-mins ahead of this body's trees
                    ctxs = []
                    for bi in range(k):
                        ctxs.append(_stream_a(bi == k - 1))
                        if bi >= 1:
                            _stream_b(ctxs[bi - 1])
                    _stream_b(ctxs[k - 1])
                    # tok3 = Copy(tok2*0 + 1) -> 1.0, ordered after the last
                    # sigma of the batch (Copy is in every table: no load)
                    nc.scalar.activation(
                        tok3[:, 0:1], tok2[:, 0:1], AF.Copy, scale=0.0, bias=PSCALE
                    )
                    for ctx in ctxs:
                        _finale(ctx)
                    # rewrite the phase token at the end of the ln phase
                    # (Copy is servable by every table -> no extra load);
                    # reading the last body's sLn stat (accum-written by its
                    # finale Ln) orders this after the finale ACT work, and
                    # scale=0 keeps the token value at 0
                    nc.scalar.activation(
                        tok[:, 0:1], ctxs[-1]["stats"][:, 3:4], AF.Copy, scale=0.0
                    )
                    i += k

            if loop_n > 1:
                u = UNROLL
                while loop_n % u:
                    u //= 2
                with tc.For_i(0, loop_n // u, 1):
                    _bodies(u)
            else:
                for _rep in range(reps):
                    _bodies(1)

    nc.compile()
    return nc


_NC_CACHE = None


def _get_nc():
    global _NC_CACHE
    if _NC_CACHE is None:
        _NC_CACHE = build_kernel_module()
    return _NC_CACHE


def kernel(s_edge: np.ndarray, s_sib: np.ndarray, mask: np.ndarray) -> np.ndarray:
    s_edge = np.ascontiguousarray(np.asarray(s_edge, dtype=np.float32))
    s_sib_bf = np.ascontiguousarray(
        np.asarray(s_sib, dtype=np.float32).astype(ml_dtypes.bfloat16)
    )
    mask_f = np.ascontiguousarray(np.asarray(mask).astype(np.float32))

    nc = _get_nc()
    in_maps = [
        {
            "ss": s_sib_bf[b],
            "se": s_edge[b].reshape(S, 2 * S),
            "mk": mask_f[b],
        }
        for b in range(B)
    ]
    res = run_bass_kernel_spmd(nc, in_maps, core_ids=list(range(B)))
    out = np.stack([res.results[b]["out"].reshape(S, S, 2) for b in range(B)])
    return out.astype(np.float32)


if __name__ == "__main__":
    rng = np.random.default_rng(0)
    se_ = rng.standard_normal((B, S, S, 2), dtype=np.float32)
    sib_ = rng.standard_normal((B, S, S, S), dtype=np.float32)
    mk_ = np.ones((B, S, S), dtype=bool)
    print(kernel(se_, sib_, mk_).shape)


# revision 45
# speedup vs baseline: 1.0727x; 1.0116x over previous
"""Trainium2 Bass kernel for nn_LoopyBeliefPropagation (B=8, S=128, 3 BP iters).

Math: the reference's loopy-BP collapses algebraically (see kernel_baseline
derivation): the only O(S^3) work is the masked softplus row reduction

    C(i,j) = sum_k softplus(s_sib[b,j,i,k]) * valid(k)

and everything else is O(S^2) per batch.  This version refactors the softplus
reduction around TWO structural changes vs the exp-space baseline:

1. bf16 streaming.  s_sib is quantized to bf16 on the host, halving the HBM
   stream from 25.3us to 12.6us per body (cost model 0.3855 ns/B/partition).
   Output-scale is ~6e3 and the absmax budget at rel 2e-3 is ~12, so the
   ~0.4% input quantization noise (sqrt-accumulated through two ~100-term
   masked sums) is far inside the budget (measured: same rel-err as f32).

2. sigmoid-space softplus:  softplus(x) = -ln sigmoid(-x).  The HW sigmoid
   table is exact at bf16 resolution (probed), so one ACT pass produces
   s_k = sigmoid(-x_k) and the masked sum becomes

    C(i,j) = -sum_k ln s_k = -ln prod s_k      (masked k contribute s_k = 1)

   This deletes the exp-space scheme's "+1" DVE pass (tensor_scalar 4x,
   4.3us/body) entirely: the product tree runs directly on sigma values.
   Masking folds into one half-width DVE min on the INPUT (lens >= S/2, so
   only k in [S/2,S) is data-dependent): min(x, valid*120-60) drives masked
   lanes to x=-60 where sigmoid(60) saturates to exactly 1.0 (probed); the
   always-invalid k=0 column is a Pool-engine memset of -60.

   Group products of 16 sigmas sit near the bottom of bf16 range, and the
   Ln table is only accurate for inputs in [1e-15, 1e15] (probed), so the
   Ln pass applies a 2^60 prescale through its scale operand (carried by
   the tok3 gating token, value 2^60): Ln(p16 * 2^60) lands in [1e8, 1e17]
   (probed on the real data).  The 8*60*ln2 offset folds into G2.

   Sigmoid and Ln live in DIFFERENT ACT tables (sigmoid_and_others vs
   natural_log_exp_and_others; the pwp softplus slot is opaque 'act2'), and
   a table load is 1283ns, so bodies are processed in batches of K=8:
   all sigma passes of the batch first (sigmoid table), then all Ln/finale
   passes (natural_log_exp table, which also serves the finale's Exp/Ln/Abs)
   -> exactly 2 table loads per batch, 321ns/body amortized.

Sign bookkeeping: the PE transpose of LnS = sum_g ln p16 is NOT negated;
instead the finale works with F = -E = (LnS - G2)*V and the stats algebra is
flipped: sE = -sF, sRelu(E) = sReluF - sF, so sP/sD/b3 come out identically.

Measured (A/B device timing): 28750ns (f32 exp baseline) -> 19804ns.
HW ablations show the kernel is DVE-bound with ~150ns real per-instruction
overhead (removing the whole sigma pass saves only ~0.5k, removing all
chunk DMAs only ~0.4k), so the structure minimizes DVE instruction count:
2 chunks of 64 feeding one body-wide sigma tile (body-wide 4-instr tree),
mask-min via a broadcast AP (materializing the replica measured slower),
PSUM->SBUF copies on ACT, small DMAs on the SP queue (each DMA costs its
issuing engine's sequencer ~600ns), and all mask/sigma DVE work issued
BEFORE tree work so the in-order DVE queue never gates ACT.

Timed via For_i with UNROLL=32 (4 batches of 8); input-independent constants
(identity, ones, zeros) are hoisted out of the loop (a real kernel launch
builds them once); all per-input work stays inside each body.

Sharding: data-parallel over batch, one batch per NeuronCore (8 cores).
"""

import numpy as np
import ml_dtypes

import concourse.bass as bass
import concourse.bacc as bacc
import concourse.tile as tile
from concourse import mybir
from concourse.bass_utils import run_bass_kernel_spmd
from concourse.masks import make_identity

B, S = 8, 128
H = S // 2
LOG2 = float(np.log(2.0))
FP32 = mybir.dt.float32
BF16 = mybir.dt.bfloat16
FP16 = mybir.dt.float16
AF = mybir.ActivationFunctionType
OP = mybir.AluOpType

GI = 64            # max i-slab per DMA chunk
SIZES = [64, 64]
OFFS = [0, 64]
SCALE_P = 60       # product prescale 2^SCALE_P at the last tree level
PSCALE = float(2.0 ** SCALE_P)
GOFF = 8 * SCALE_P * LOG2   # ln-offset collected by the 8 groups per row
K = 8              # bodies per ACT-table batch
UNROLL = 32


def _pin_act_tables():
    """Restrict activation-table choice to the two sets this kernel needs:
    sigmoid_and_others (the sigma pass) and natural_log_exp_and_others
    (chunk Ln + the finale's Abs/Exp/Ln/Relu).  Pinning prevents Bacc's
    table-load pass from picking a third set (e.g. exp_and_others for the
    finale Exp), which would break the 2-loads-per-batch schedule.  Set ids
    are positional, so other entries are emptied rather than removed."""
    import concourse.hw_specs as hw_specs

    if getattr(hw_specs.get_activation_tables, "_bp_pinned", False):
        return
    orig = hw_specs.get_activation_tables

    KEEP = ("sigmoid_and_others", "natural_log_exp_and_others")

    def pinned(module_arch):
        tables = orig(module_arch)
        return {
            name: (funcs if name in KEEP else set())
            for name, funcs in tables.items()
        }

    pinned._bp_pinned = True
    hw_specs.get_activation_tables = pinned
    import concourse.bacc as _bacc_mod

    if getattr(_bacc_mod, "get_activation_tables", None) is orig:
        _bacc_mod.get_activation_tables = pinned


def build_kernel_module(reps: int = 1, loop_n: int = 0, variant: str = "full"):
    _pin_act_tables()
    nc = bacc.Bacc("TRN2", debug=False, target_bir_lowering=False)

    ss = nc.dram_tensor("ss", [S, S, S], BF16, kind="ExternalInput")   # s_sib[b] (j,i,k) bf16
    se = nc.dram_tensor("se", [S, 2 * S], FP32, kind="ExternalInput")  # s_edge[b] (j, i*2+q)
    mk = nc.dram_tensor("mk", [S, S], FP32, kind="ExternalInput")      # mask[b] as f32
    out = nc.dram_tensor("out", [S, 2 * S], FP32, kind="ExternalOutput")

    with tile.TileContext(nc) as tc:
        with (
            tc.tile_pool(name="fixed", bufs=1) as fixed,
            tc.tile_pool(name="consts", bufs=K) as consts,
            tc.tile_pool(name="coll", bufs=K) as collp,
            tc.tile_pool(name="small", bufs=3) as small,
            tc.tile_pool(name="chunks", bufs=3) as chunks,
            tc.tile_pool(name="spp", bufs=2) as spp,
            tc.tile_pool(name="mxp", bufs=1) as mxp,
            tc.tile_pool(name="mp1", bufs=1) as mp1,
            tc.tile_pool(name="mp2", bufs=1) as mp2,
            tc.tile_pool(name="mp3", bufs=1) as mp3,
            tc.tile_pool(name="lpp", bufs=2) as lpp,
            tc.tile_pool(name="scratch", bufs=2) as scratch,
            tc.tile_pool(name="psum", bufs=1, space="PSUM") as psum,
        ):
            # ---- input-independent constants, hoisted out of the loop ----
            ident = fixed.tile([S, S], FP32)
            make_identity(nc, ident)
            ones1 = fixed.tile([1, S], FP32)
            nc.vector.memset(ones1[:], 1.0)
            zeros = fixed.tile([S, S], FP32)
            nc.gpsimd.memset(zeros[:], 0.0)
            # tok (always 0.0) serializes ACT table phases: every sigma pass
            # reads it as bias, and it is rewritten by a Copy at the end of
            # each batch's natural_log phase, so the scheduler cannot slide
            # next-batch sigmas into this batch's finale (table thrash)
            tok = fixed.tile([S, 1], FP32)
            nc.vector.memset(tok[:], 0.0)
            # tok2 collects the batch's last sigma accum (value unused);
            # tok3 = Copy(tok2*0 + 1) == 1.0 gates every body-Ln's scale so
            # no Ln can be scheduled before the batch's sigmas finish
            tok2 = fixed.tile([S, 1], FP32)
            nc.vector.memset(tok2[:], 0.0)
            tok3 = fixed.tile([S, 1], FP32)

            def _stream_a(last_in_batch):
                # ---- part A: DMAs, mask-min, sigma passes, consts ----
                # flat 2D APs on both sides: the (i,k) dims are contiguous
                # in DRAM and SBUF, and a [S, gi*S] view gives 12KB runs
                # (3D [S,gi,S] APs have 256B innermost rows, under the 512B
                # threshold where the DMA pays a ~2x latency multiplier)
                ss2d = ss[:].rearrange("p i k -> p (i k)")
                cks = []
                for c in range(len(SIZES)):
                    ck = chunks.tile([S, GI, S], BF16, name="chunk")
                    ck2d = ck[:].rearrange("p i k -> p (i k)")
                    if variant != "nodma":
                        q = nc.sync if (variant != "twoq" or c % 2 == 0) else nc.vector
                        q.dma_start(
                            out=ck2d[:, : SIZES[c] * S],
                            in_=ss2d[:, OFFS[c] * S : (OFFS[c] + SIZES[c]) * S],
                        )
                    cks.append(ck)

                V = consts.tile([S, S], FP32)
                nc.sync.dma_start(out=V, in_=mk[:])
                vkrow = consts.tile([1, H], FP32)
                nc.sync.dma_start(out=vkrow, in_=mk[1:2, H:])
                se_sb = small.tile([S, 2 * S], FP32)
                nc.sync.dma_start(out=se_sb, in_=se[:])

                # hi-half mask row -> min-mask Mx = valid*120-60 (+-60),
                # broadcast to all partitions by a rank-1 matmul; the min
                # consumes it as a stride-0-middle broadcast AP (2x packing
                # holds; a materialized replica measured slower on HW)
                vk_ps = psum.tile([S, H], FP32, tag="vk_ps")
                nc.tensor.matmul(vk_ps[:], ones1[:], vkrow[:], start=True, stop=True)
                Mxr = consts.tile([S, H], BF16)
                nc.vector.tensor_scalar(
                    out=Mxr[:], in0=vk_ps[:], scalar1=120.0, scalar2=-60.0,
                    op0=OP.mult, op1=OP.add,
                )
                # broadcast AP straight into the min: the materialized
                # replica copy measured SLOWER on HW (the 2x_1p packing
                # holds with a stride-0 middle dim; innermost stays packed)
                MxRep = Mxr[:, None, :].broadcast_to([S, GI, H])

                # mask + sigma per chunk, issued BEFORE any tree work so
                # the in-order DVE queue never gates the ACT sigma stream;
                # both chunks' sigmas land in ONE body tile so the product
                # tree below runs body-wide (4 DVE instrs, not 8)
                sigbody = spp.tile([S, S, S], BF16, name="sigbody")
                for c in range(len(SIZES)):
                    gi, i0 = SIZES[c], OFFS[c]
                    chunk = cks[c]
                    if variant != "nomin":
                        nc.vector.tensor_tensor(
                            chunk[:, :gi, H:], chunk[:, :gi, H:], MxRep[:, :gi],
                            OP.min,
                        )
                        nc.gpsimd.memset(chunk[:, :gi, 0:1], -60.0)
                    accum = (
                        dict(accum_out=tok2[:, 0:1])
                        if (last_in_batch and c == len(SIZES) - 1)
                        else {}
                    )
                    if variant != "nosigma":
                        nc.scalar.activation(
                            sigbody[:, i0 : i0 + gi, :], chunk[:, :gi, :],
                            AF.Sigmoid, scale=-1.0, bias=tok[:, 0:1], **accum,
                        )

                stats = consts.tile([S, 8], FP32)  # A,N,G2,sP,sF,sD,nsP,sReluF

                se3 = se_sb[:].rearrange("p (i q) -> p i q", q=2)
                pe0_ps = psum.tile([S, S], FP32, tag="pe0_ps")
                nc.tensor.transpose(pe0_ps[:], se3[:, :, 0], ident[:])
                pe0 = consts.tile([S, S], FP32)
                nc.scalar.activation(pe0[:], pe0_ps[:], AF.Copy)
                pe1_ps = psum.tile([S, S], FP32, tag="pe1_ps")
                nc.tensor.transpose(pe1_ps[:], se3[:, :, 1], ident[:])
                pe1 = consts.tile([S, S], FP32)
                nc.scalar.activation(pe1[:], pe1_ps[:], AF.Copy)

                Dpe = consts.tile([S, S], FP32)
                nc.vector.tensor_tensor(Dpe[:], pe1[:], pe0[:], OP.subtract)

                scr0 = scratch.tile([S, S], FP32)
                nc.vector.scalar_tensor_tensor(
                    out=scr0[:], in0=Dpe[:], scalar=1.0, in1=V[:],
                    op0=OP.mult, op1=OP.mult, accum_out=stats[:, 0:1],
                )
                nc.vector.tensor_reduce(
                    out=stats[:, 1:2], in_=V[:], axis=mybir.AxisListType.X, op=OP.add,
                )
                nc.vector.scalar_tensor_tensor(
                    out=stats[:, 2:3], in0=stats[:, 1:2], scalar=-LOG2,
                    in1=stats[:, 0:1], op0=OP.mult, op1=OP.add,
                )
                nc.vector.tensor_scalar(
                    out=stats[:, 2:3], in0=stats[:, 2:3], scalar1=GOFF,
                    scalar2=None, op0=OP.add,
                )
                return dict(V=V, stats=stats, sig=sigbody, pe0=pe0, pe1=pe1)

            def _stream_b(ctx):
                # ---- part B: ONE body-wide product tree ----
                coll = collp.tile([S, S, 8], BF16, name="coll")
                if variant == "notree":
                    nc.gpsimd.memset(coll[:], 1.0)
                else:
                    sig = ctx["sig"]
                    m1 = mp1.tile([S, S, 64], BF16)
                    nc.vector.tensor_tensor(
                        m1[:], sig[:, :, 0:64], sig[:, :, 64:128], OP.mult,
                    )
                    m2 = mp2.tile([S, S, 32], BF16)
                    nc.vector.tensor_tensor(
                        m2[:], m1[:, :, 0:32], m1[:, :, 32:64], OP.mult,
                    )
                    m3 = mp3.tile([S, S, 16], BF16)
                    nc.vector.tensor_tensor(
                        m3[:], m2[:, :, 0:16], m2[:, :, 16:32], OP.mult,
                    )
                    nc.vector.tensor_tensor(
                        coll[:], m3[:, :, 0:8], m3[:, :, 8:16], OP.mult,
                    )
                ctx["coll"] = coll

            def _finale(ctx):
                # ---- natural_log_exp-table phase of one body -------------
                V, stats = ctx["V"], ctx["stats"]
                pe0, pe1 = ctx["pe0"], ctx["pe1"]

                lnb = lpp.tile([S, S, 8], FP16, name="lnb")
                nc.scalar.activation(
                    lnb[:], ctx["coll"][:], AF.Ln, scale=tok3[:, 0:1]
                )
                # 8-wide add-reduce as a 3-level fp16 tt tree: levels 1-2
                # get 2x_1p packing that tensor_reduce never does
                a1 = lpp.tile([S, S, 4], FP16, name="a1")
                nc.vector.tensor_tensor(
                    a1[:], lnb[:, :, 0:4], lnb[:, :, 4:8], OP.add,
                )
                a2 = lpp.tile([S, S, 2], FP16, name="a2")
                nc.vector.tensor_tensor(
                    a2[:], a1[:, :, 0:2], a1[:, :, 2:4], OP.add,
                )
                LnS = lpp.tile([S, S], FP32, name="LnS")
                nc.vector.tensor_tensor(
                    LnS[:], a2[:, :, 0], a2[:, :, 1], OP.add,
                )
                lns_ps = psum.tile([S, S], FP32, tag="lns_ps", bufs=2)
                nc.tensor.transpose(lns_ps[:], LnS[:], ident[:])

                # F = -E = (LnS^T - G2)*V   (E is masked; F too)
                F = small.tile([S, S], FP32)
                nc.vector.scalar_tensor_tensor(
                    out=F[:], in0=lns_ps[:], scalar=stats[:, 2:3], in1=V[:],
                    op0=OP.subtract, op1=OP.mult,
                )

                # stable softplus row sums of E = -F:
                #   sLn = sum Ln(1+Exp(-|F|)), sReluF = sum relu(F), sF = sum F
                #   sP = sLn + sReluF - sF + log2*N - S*log2 ; sE = -sF
                aE = small.tile([S, S], FP32)
                nc.scalar.activation(aE[:], F[:], AF.Abs)
                nc.scalar.activation(aE[:], aE[:], AF.Exp, scale=-1.0)
                lnp = scratch.tile([S, S], FP32)
                nc.scalar.activation(
                    lnp[:], aE[:], AF.Ln, bias=1.0, accum_out=stats[:, 3:4]
                )
                nc.vector.tensor_reduce(
                    out=stats[:, 4:5], in_=F[:], axis=mybir.AxisListType.X, op=OP.add,
                )
                # relscr = relu(F) - log2 per element, so its row sum is
                # sReluF - S*log2 and the -S*log2 term costs no extra instr
                relscr = scratch.tile([S, S], FP32)
                nc.vector.tensor_scalar(
                    out=relscr[:], in0=F[:], scalar1=0.0, scalar2=-LOG2,
                    op0=OP.max, op1=OP.add,
                )
                nc.vector.tensor_reduce(
                    out=stats[:, 7:8], in_=relscr[:], axis=mybir.AxisListType.X,
                    op=OP.add,
                )
                # t = sLn + (sReluF - S*log2) + log2*N = sP + sF
                # nsP = sF - t ; sD = nsP - sF
                nc.vector.tensor_tensor(
                    stats[:, 3:4], stats[:, 3:4], stats[:, 7:8], OP.add
                )
                nc.vector.scalar_tensor_tensor(
                    out=stats[:, 3:4], in0=stats[:, 1:2], scalar=LOG2,
                    in1=stats[:, 3:4], op0=OP.mult, op1=OP.add,
                )
                nc.vector.tensor_tensor(
                    stats[:, 6:7], stats[:, 4:5], stats[:, 3:4], OP.subtract
                )
                nc.vector.tensor_tensor(
                    stats[:, 5:6], stats[:, 6:7], stats[:, 4:5], OP.subtract
                )

                # b3_0 = (pe0 - sP) * V ; b3_1 = (pe1 + sD) * V
                b30 = small.tile([S, S], FP32)
                nc.vector.scalar_tensor_tensor(
                    out=b30[:], in0=pe0[:], scalar=stats[:, 6:7], in1=V[:],
                    op0=OP.add, op1=OP.mult,
                )
                b31 = small.tile([S, S], FP32)
                nc.vector.scalar_tensor_tensor(
                    out=b31[:], in0=pe1[:], scalar=stats[:, 5:6], in1=V[:],
                    op0=OP.add, op1=OP.mult,
                )

                t0_ps = psum.tile([S, S], FP32, tag="t0_ps")
                nc.tensor.transpose(t0_ps[:], b30[:], ident[:])
                t1_ps = psum.tile([S, S], FP32, tag="t1_ps")
                nc.tensor.transpose(t1_ps[:], b31[:], ident[:])

                outT = small.tile([S, 2 * S], FP32)
                out3 = outT[:].rearrange("p (i q) -> p i q", q=2)
                nc.scalar.activation(out3[:, :, 0], t0_ps[:], AF.Copy)
                nc.scalar.activation(out3[:, :, 1], t1_ps[:], AF.Copy)
                nc.sync.dma_start(out=out[:], in_=outT)

            def _bodies(n):
                # batches of K bodies: all sigma-table work first, then all
                # natural_log-table work -> 2 ACT table loads per batch.
                # The batching is enforced STRUCTURALLY (the tile scheduler
                # does not preserve ACT program order): one bank-wide Ln
                # depends on every body's sigma chain, and next-batch sigmas
                # read `tok`, rewritten at the end of this batch's ln phase.
                i = 0
                while i < n:
                    k = min(K, n - i)
                    # A0 A1 B0 A2 B1 ... : body x's trees (B) issue after
                    # body x+1's sigmas (A) so the in-order DVE queue keeps
                    # the next body's mask-mins ahead of this body's trees
                    ctxs = []
                    for bi in range(k):
                        ctxs.append(_stream_a(bi == k - 1))
                        if bi >= 1:
                            _stream_b(ctxs[bi - 1])
                    _stream_b(ctxs[k - 1])
                    # tok3 = Copy(tok2*0 + 1) -> 1.0, ordered after the last
                    # sigma of the batch (Copy is in every table: no load)
                    nc.scalar.activation(
                        tok3[:, 0:1], tok2[:, 0:1], AF.Copy, scale=0.0, bias=PSCALE
                    )
                    for ctx in ctxs:
                        _finale(ctx)
                    # rewrite the phase token at the end of the ln phase
                    # (Copy is servable by every table -> no extra load);
                    # reading the last body's sLn stat (accum-written by its
                    # finale Ln) orders this after the finale ACT work, and
                    # scale=0 keeps the token value at 0
                    nc.scalar.activation(
                        tok[:, 0:1], ctxs[-1]["stats"][:, 3:4], AF.Copy, scale=0.0
                    )
                    i += k

            if loop_n > 1:
                u = UNROLL
                while loop_n % u:
                    u //= 2
                with tc.For_i(0, loop_n // u, 1):
                    _bodies(u)
            else:
                for _rep in range(reps):
                    _bodies(1)

    nc.compile()
    return nc


_NC_CACHE = None


def _get_nc():
    global _NC_CACHE
    if _NC_CACHE is None:
        _NC_CACHE = build_kernel_module()
    return _NC_CACHE


def kernel(s_edge: np.ndarray, s_sib: np.ndarray, mask: np.ndarray) -> np.ndarray:
    s_edge = np.ascontiguousarray(np.asarray(s_edge, dtype=np.float32))
    s_sib_bf = np.ascontiguousarray(
        np.asarray(s_sib, dtype=np.float32).astype(ml_dtypes.bfloat16)
    )
    mask_f = np.ascontiguousarray(np.asarray(mask).astype(np.float32))

    nc = _get_nc()
    in_maps = [
        {
            "ss": s_sib_bf[b],
            "se": s_edge[b].reshape(S, 2 * S),
            "mk": mask_f[b],
        }
        for b in range(B)
    ]
    res = run_bass_kernel_spmd(nc, in_maps, core_ids=list(range(B)))
    out = np.stack([res.results[b]["out"].reshape(S, S, 2) for b in range(B)])
    return out.astype(np.float32)


if __name__ == "__main__":
    rng = np.random.default_rng(0)
    se_ = rng.standard_normal((B, S, S, 2), dtype=np.float32)
    sib_ = rng.standard_normal((B, S, S, S), dtype=np.float32)
    mk_ = np.ones((B, S, S), dtype=bool)
    print(kernel(se_, sib_, mk_).shape)
